# revision 7
# baseline (speedup 1.0000x reference)
"""GAT layer kernel v3 for Trainium2 — nn_Basic_GAT_80874234184376.

Sharding (8 cores): graph b = core//2, query-row half i0 = (core%2)*512.

Per core (i: 512 query rows, j: 1024 keys, h: 16 heads, chunk: 8 keys):
  - edge pre-transposed + bf16 on HOST to [jb, (j8,f), cc, i]; 1 DMA/j-block.
  - psl[(h,j8), i] = CMB matmul (mask -1e9 rows + att1 spread rows against a
    staged [24,16,512] rhs of nadj-chunk rows and replicated att1T)
    += BD matmul (block-diag ae_w) @ edge-slab slice (att_e).
    att2 + biases + att_g ride in as per-partition activation bias.
  - softmax-invariant scaling by 100: E = max(100*exp(y), y+100)
    = ACT Exp(bias+ln100) then ONE fused DVE scalar_tensor_tensor.
  - U[i, 272] += E.T @ vbig (resident values + ones cols, bf16), 2-chunk lag.
  - epilogue: U/den, + skip matmul + skip_b, relu, layernorm; DVE/Pool split.
HWDGE descriptor gen is ~630ns serial per DMA -> constants packed into two
host buffers (1 DMA each), single-DMA gathers for att2/vald roundtrips.
"""

import numpy as np

B, N, FN, FE, FG = 4, 1024, 128, 16, 128
OUT, H = 256, 16
HD = OUT // H
NCORES = 8
ROWS = N // 2          # query rows per core
NIB = ROWS // 128      # 4 i-blocks
NJB = N // 8 // 16     # 8 j-blocks
NCC = 16               # 8-key chunks per j-block
NCH = NJB * NCC        # 128 chunks total

import os as _os
DBG_NJB = int(_os.environ.get("DBG_NJB", str(NJB)))
DBG_NO_EPI = _os.environ.get("DBG_NO_EPI", "0") == "1"

# packed bf16 const buffer column offsets
C_NODET, C_NOWN, C_MW, C_SW = 0, 1024, 1536, 1792
C_A12, C_AGW, C_BD, C_CMB, C_TI, C_ONES = 2048, 2080, 2096, 2224, 2352, 2480
C_TI8 = 2496
C_GT = 2496 + 8 * 128
CBF_COLS = C_GT + 1
# packed f32 const buffer column offsets
F_MB, F_SB, F_LNS, F_LNO, F_HB, F_GT, F_EPS = 0, 256, 512, 768, 1024, 1025, 1026
CF32_COLS = 1027

_CACHED = {}


def _patched_tc(nc):
    """TileContext whose tail drain splits multi-sem waits (this walrus build
    allows at most one sem wait per non-EVSEM instruction)."""
    import concourse.mybir as mybir
    from concourse.tile import TileContext
    from concourse.vector_clock import ScopedClock

    class TC(TileContext):
        def _drain_and_barrier(self, tick_clock, wait_clock):
            ncc = self.nc
            drain_inst = ncc.sync.drain()
            wait_clock.add_sem_waits(
                drain_inst.ins, ScopedClock({None: tick_clock.global_clock}))
            si = drain_inst.ins.sync_info
            waits = list(si.on_wait) if si is not None else []
            if len(waits) > 1:
                drain_inst.ins.sync_info = mybir.SyncInfo(
                    on_wait=[waits[0]], on_update=list(si.on_update))
                for w in waits[1:]:
                    extra = ncc.sync.drain()
                    extra.ins.sync_info = mybir.SyncInfo(on_wait=[w], on_update=[])
            ncc.all_engine_barrier()
            popped = ncc._tile_sem_poison_stack.pop()
            assert popped is self._sem_poison
            ncc.clear_and_free_semaphores(list(self.sems.allocated().values()))
            ncc.all_engine_barrier()

    return TC(nc)


def _build_module():
    import concourse.bacc as bacc
    import concourse.mybir as mybir

    f32 = mybir.dt.float32
    bf16 = mybir.dt.bfloat16
    fp8 = mybir.dt.float8e4
    AL = mybir.AluOpType
    ACT = mybir.ActivationFunctionType

    nc = bacc.Bacc("TRN2", num_devices=NCORES)

    edge_d = nc.dram_tensor("edge", (NJB, 128, NCC, ROWS), fp8,
                            kind="ExternalInput")
    nadj2_d = nc.dram_tensor("nadj2", (8, NCH, ROWS), bf16,
                             kind="ExternalInput")
    cbf_d = nc.dram_tensor("cbf", (128, CBF_COLS), bf16, kind="ExternalInput")
    cf32_d = nc.dram_tensor("cf32", (128, CF32_COLS), f32,
                            kind="ExternalInput")
    out_d = nc.dram_tensor("out", (ROWS, OUT), f32, kind="ExternalOutput")

    vald = nc.dram_tensor("vald", (N, OUT), bf16)      # internal scratch
    att1d = nc.dram_tensor("att1d", (H, ROWS), bf16)   # internal scratch

    with _patched_tc(nc) as tc:
        with (
            tc.tile_pool(name="const", bufs=1) as cpool,
            tc.tile_pool(name="work", bufs=2) as wpool,
            tc.tile_pool(name="ew", bufs=4) as ewpool,
            tc.tile_pool(name="epool", bufs=8) as eppool,
            tc.tile_pool(name="upsum", bufs=1, space="PSUM") as upool,
        ):
            # ---- bulk const loads (2 DMAs) + slabs + nadj (3 DMAs) -------
            cbf = cpool.tile([128, CBF_COLS], bf16)
            nc.sync.dma_start(cbf[:], cbf_d[:, :])
            cf = cpool.tile([128, CF32_COLS], f32)
            nc.sync.dma_start(cf[:], cf32_d[:, :])
            esbs = [cpool.tile([128, NCC, ROWS], fp8, name=f"esb{k}")
                    for k in range(3)]
            nc.sync.dma_start(esbs[0][:], edge_d[0])

            nodeT = cbf[:, C_NODET:C_NODET + N]
            nown = cbf[:, C_NOWN:C_NOWN + ROWS]
            mw_sb = cbf[:, C_MW:C_MW + OUT]
            sw_sb = cbf[:, C_SW:C_SW + OUT]
            a12_sb = cbf[:, C_A12:C_A12 + 2 * H]
            agw_sb = cbf[:, C_AGW:C_AGW + H]
            bd_sb = cbf[:, C_BD:C_BD + 128]
            cmb_sb = cbf[0:24, C_CMB:C_CMB + 128]
            ti_sb = cbf[0:16, C_TI:C_TI + 128]
            ones_sb = cbf[:, C_ONES:C_ONES + H]
            mb_sb = cf[:, F_MB:F_MB + OUT]
            sb_sb = cf[:, F_SB:F_SB + OUT]
            lns_sb = cf[:, F_LNS:F_LNS + OUT]
            lno_sb = cf[:, F_LNO:F_LNO + OUT]
            hb_sb = cf[:, F_HB:F_HB + 1]
            gt_sb = cf[:, F_GT:F_GT + 1]
            epst = cf[:, F_EPS:F_EPS + 1]

            # ---- vbig: double-buffered per-j-block values ----------------
            VG = 4
            vbufs = [cpool.tile([128, VG, NCC, OUT + H], bf16, name=f"vb{k}")
                     for k in range(2)]
            # zeros+ones: buf0 on ACT+DVE (needed early), buf1 on Pool
            nc.scalar.activation(
                vbufs[0][:].rearrange("p g b c -> p (g b c)"),
                ones_sb[:, 0:1].broadcast_to((128, VG * NCC * (OUT + H))),
                ACT.Identity, scale=0.0)
            nc.vector.tensor_copy(
                vbufs[0][:, :, :, OUT:OUT + H],
                ones_sb[:, None, None, :].broadcast_to((128, VG, NCC, H)))
            nc.vector.memset(vbufs[1][:, 0:2], 0.0)
            nc.gpsimd.memset(vbufs[1][:, 2:4], 0.0)
            nc.gpsimd.tensor_copy(
                vbufs[1][:, :, :, OUT:OUT + H],
                ones_sb[:, None, None, :].broadcast_to((128, VG, NCC, H)))

            def refill_vbig(g, split=False):
                for h in range(H):
                    nc.sync.dma_start(
                        vbufs[g % 2][8 * h:8 * h + 8, :, :,
                                     16 * h:16 * (h + 1)],
                        vald[g * VG * 128:(g + 1) * VG * 128,
                             16 * h:16 * (h + 1)]
                        .rearrange("(g c j8) hd -> j8 g c hd", j8=8, c=NCC))

            att1T = cpool.tile([H, ROWS], bf16)
            att2e = cpool.tile([128, NCH], f32)   # att2 bias + ln(100)
            att2l = cpool.tile([128, NCH], f32)   # att2 bias + 100
            stags = [cpool.tile([24, NCC, ROWS], bf16, name=f"stag{k}")
                     for k in range(3)]

            with (tc.tile_pool(name="p0ps", bufs=1, space="PSUM") as p0ps,
                  tc.tile_pool(name="p0d", bufs=2, space="PSUM") as p0d):
                # att_g: (graph @ ag_w) spread to 128 partitions + host biases
                gps = p0ps.tile([128, 512], f32, tag="pS", name="gps")
                nc.tensor.matmul(gps[0:H, 0:1], agw_sb[:],
                                 cbf[:, C_GT:C_GT + 1], start=True,
                                 stop=True)
                gsb = wpool.tile([H, 1], bf16, tag="gsb")
                nc.vector.tensor_copy(gsb[:], gps[0:H, 0:1])
                gx = p0ps.tile([128, 512], f32, tag="pS", name="gx")
                nc.tensor.matmul(gx[:, 0:1], ti_sb, gsb[:], start=True, stop=True)
                hb128 = cpool.tile([128, 1], f32)
                nc.vector.tensor_tensor(hb128[:], gx[:, 0:1], hb_sb, AL.add)

                # att1T directly transposed: [h, i] = a1_w.T @ nownT
                a1ps = p0ps.tile([128, ROWS], f32, tag="pS", name="a1ps")
                nc.tensor.matmul(a1ps[0:H, :], a12_sb[:, 0:H], nown[:],
                                 start=True, stop=True)
                nc.vector.tensor_copy(att1T[:], a1ps[0:H, :])
                nc.sync.dma_start(att1d[:, :], att1T[:])
                for k in range(3):
                    nc.sync.dma_start(
                        stags[k][8:24, :, :],
                        att1d[:, None, :].broadcast_to((H, NCC, ROWS)))
                for k in range(min(2, DBG_NJB)):
                    nc.sync.dma_start(stags[k][0:8, :, :],
                                      nadj2_d[:, k * NCC:(k + 1) * NCC, :])

                # att2T directly transposed: [h, j] = a2_w.T @ nodeT
                att2T = cpool.tile([H, N], bf16)
                for half in range(2):
                    a2ps = p0ps.tile([128, ROWS], f32, tag="pS",
                                     name=f"a2ps{half}")
                    nc.tensor.matmul(a2ps[0:H, :],
                                     a12_sb[:, H:2 * H],
                                     nodeT[:, half * ROWS:(half + 1) * ROWS],
                                     start=True, stop=True)
                    nc.vector.tensor_copy(
                        att2T[:, half * ROWS:(half + 1) * ROWS], a2ps[0:H, :])
                # partition-spread att2[j, h] -> a2xps[(h,j8), cg] in PSUM
                a2xps = p0ps.tile([128, NCH], f32, tag="pC", name="a2xps")
                for j8 in range(8):
                    nc.tensor.matmul(
                        a2xps[:],
                        cbf[0:16, C_TI8 + 128 * j8:C_TI8 + 128 * (j8 + 1)],
                        att2T[:, :].rearrange("h (cg j8) -> h cg j8",
                                              j8=8)[:, :, j8],
                        start=(j8 == 0), stop=(j8 == 7))
                a2xp = wpool.tile([128, NCH], f32, tag="a2xp")
                nc.vector.tensor_scalar(a2xp[:], a2xps[:], hb128[:, 0:1], None,
                                        AL.add)
                nc.vector.tensor_scalar(att2e[:], a2xp[:], 4.605170185988091,
                                        None, AL.add)
                nc.vector.tensor_scalar(att2l[:], a2xp[:], 100.0, None,
                                        AL.add)

                # values = node @ m_w + (m_b + skip_b bias) -> vald
                vall = cpool.tile([128, 8, OUT], bf16)
                for jt in range(8):
                    vps = p0d.tile([128, OUT], f32, tag="pD", name=f"v{jt}")
                    nc.tensor.matmul(vps[:], nodeT[:, jt * 128:(jt + 1) * 128],
                                     mw_sb[:], start=True, stop=True)
                    nc.vector.tensor_tensor(vall[:, jt, :], vps[:], mb_sb[:],
                                            AL.add)
                nc.sync.dma_start(
                    vald[:, :].rearrange("(jt p) c -> p jt c", jt=8),
                    vall[:])
                for h in range(H):
                    eng_q = nc.sync if h % 2 else nc.gpsimd
                    eng_q.dma_start(
                        vbufs[0][8 * h:8 * h + 8, :, :, 16 * h:16 * (h + 1)],
                        vald[0:VG * 128, 16 * h:16 * (h + 1)]
                        .rearrange("(g c j8) hd -> j8 g c hd", j8=8, c=NCC))
                if DBG_NJB > 1:
                    nc.sync.dma_start(esbs[1][:], edge_d[1])
                if DBG_NJB > 2:
                    nc.sync.dma_start(esbs[2][:], edge_d[2])
                sksb = cpool.tile([128, NIB, OUT], bf16)
                for ib in range(NIB):
                    skp = p0d.tile([128, OUT], f32, tag="pD", name=f"sk{ib}")
                    nc.tensor.matmul(skp[:], nown[:, ib * 128:(ib + 1) * 128],
                                     sw_sb[:], start=True, stop=True)
                    nc.vector.tensor_copy(sksb[:, ib, :], skp[:])

            # U accumulators: [i(128), 256 vals + 16 den] per i-block
            upsums = [upool.tile([128, OUT + H], f32, tag=f"U{ib}",
                                 name=f"U{ib}") for ib in range(NIB)]

            # ---------------- main loop ----------------
            LAG = 2
            WARM = min(int(_os.environ.get('WARM', '2')), max(2, DBG_NJB * NCC - 2))
            NCG = DBG_NJB * NCC
            etiles = {}
            unext = [0]

            def emit_u(cg2):
                jb2, cc2 = cg2 // NCC, cg2 % NCC
                E2 = etiles.pop(cg2)
                for ib in range(NIB):
                    nc.tensor.matmul(
                        upsums[ib][:],
                        E2[:, ib * 128:(ib + 1) * 128],
                        vbufs[(jb2 // VG) % 2][:, jb2 % VG, cc2, :],
                        start=(cg2 == 0), stop=(cg2 == NCG - 1))

            with tc.tile_pool(name="pslp", bufs=4, space="PSUM") as pslpool:
                for jb in range(DBG_NJB):
                    esb = esbs[jb % 3]
                    stag = stags[jb % 3]
                    for cc in range(NCC):
                        cg = jb * NCC + cc
                        if cg >= WARM + LAG:
                            n = 2 if unext[0] < cg - LAG - 1 else 1
                            for _ in range(n):
                                if unext[0] <= cg - LAG - 1:
                                    emit_u(unext[0])
                                    unext[0] += 1
                        psl = pslpool.tile([128, ROWS], f32, tag="psl")
                        nc.tensor.matmul(psl[:], cmb_sb, stag[:, cc, :],
                                         start=True, stop=False)
                        nc.tensor.matmul(psl[:], bd_sb, esb[:, cc, :],
                                         start=False, stop=True)
                        e0 = ewpool.tile([128, ROWS], bf16, tag="e0")
                        nc.scalar.activation(e0[:], psl[:], ACT.Exp,
                                             bias=att2e[:, cg:cg + 1],
                                             scale=1.0)
                        E = eppool.tile([128, ROWS], bf16, tag="E")
                        nc.vector.scalar_tensor_tensor(
                            E[:], psl[:], att2l[:, cg:cg + 1], e0[:],
                            AL.add, AL.max)
                        etiles[cg] = E
                        if cc == 2 and jb % VG == 1 and (jb // VG) + 1 < (DBG_NJB + VG - 1) // VG:
                            refill_vbig(jb // VG + 1)

                    if jb + 2 < DBG_NJB:
                        nc.sync.dma_start(
                            stags[(jb + 2) % 3][0:8, :, :],
                            nadj2_d[:, (jb + 2) * NCC:(jb + 3) * NCC, :])
                    if jb + 3 < DBG_NJB:
                        nc.sync.dma_start(esbs[(jb + 3) % 3][:],
                                          edge_d[jb + 3])
                for cg in range(unext[0], NCG):
                    emit_u(cg)

            # ---------------- epilogue ----------------
            oall = cpool.tile([128, NIB, OUT], f32)
            NEP = 0 if DBG_NO_EPI else NIB
            recs = [wpool.tile([128, H, 1], f32, tag=f"rec{i}", name=f"rec{i}") for i in range(NEP)]
            unos = [wpool.tile([128, OUT], bf16, tag=f"uno{i}", name=f"uno{i}") for i in range(NEP)]
            mus = [wpool.tile([128, 1], f32, tag=f"mu{i}", name=f"mu{i}") for i in range(NEP)]
            cens = [wpool.tile([128, OUT], bf16, tag=f"cen{i}", name=f"cen{i}") for i in range(NEP)]
            sqs = [wpool.tile([128, OUT], bf16, tag=f"sq{i}", name=f"sq{i}") for i in range(NEP)]
            vars_ = [wpool.tile([128, 1], f32, tag=f"var{i}", name=f"var{i}") for i in range(NEP)]
            stds = [wpool.tile([128, 1], f32, tag=f"std{i}", name=f"std{i}") for i in range(NEP)]
            rstds = [wpool.tile([128, 1], f32, tag=f"rst{i}", name=f"rst{i}") for i in range(NEP)]
            engs = [nc.vector if i == 3 else nc.gpsimd for i in range(NEP)]
            for ib in range(NEP):
                nc.vector.reciprocal(recs[ib][:, :, 0],
                                     upsums[ib][:, OUT:OUT + H])
            for ib in range(NEP):
                nc.vector.tensor_tensor(
                    unos[ib][:].rearrange("p (h d) -> p h d", h=H),
                    upsums[ib][:, 0:OUT].rearrange("p (h d) -> p h d", h=H),
                    recs[ib][:, :, :].broadcast_to((128, H, HD)),
                    AL.mult)
            for ib in range(NEP):
                engs[ib].tensor_tensor(unos[ib][:], unos[ib][:],
                                       sksb[:, ib, :], AL.add)
            for ib in range(NEP):
                engs[ib].tensor_scalar(unos[ib][:], unos[ib][:], 0.0, None,
                                       AL.max)
            for ib in range(NEP):
                nc.vector.reduce_sum(mus[ib][:], unos[ib][:],
                                     axis=mybir.AxisListType.X)
            for ib in range(NEP):
                engs[ib].tensor_scalar(mus[ib][:], mus[ib][:], 1.0 / OUT,
                                       None, AL.mult)
            for ib in range(NEP):
                nc.vector.tensor_scalar(cens[ib][:], unos[ib][:],
                                        mus[ib][:, 0:1], None, AL.subtract)
            for ib in range(NEP):
                engs[ib].tensor_tensor(sqs[ib][:], cens[ib][:], cens[ib][:],
                                       AL.mult)
            for ib in range(NEP):
                nc.vector.reduce_sum(vars_[ib][:], sqs[ib][:],
                                     axis=mybir.AxisListType.X)
            for ib in range(NEP):
                nc.scalar.activation(stds[ib][:], vars_[ib][:], ACT.Sqrt,
                                     bias=epst, scale=1.0 / OUT)
            for ib in range(NEP):
                nc.vector.reciprocal(rstds[ib][:], stds[ib][:])
            for ib in range(NEP):
                nc.vector.scalar_tensor_tensor(cens[ib][:], cens[ib][:],
                                               rstds[ib][:, 0:1], lns_sb,
                                               AL.mult, AL.mult)
            for ib in range(NEP):
                engs[ib].tensor_tensor(oall[:, ib, :], cens[ib][:], lno_sb,
                                       AL.add)
            nc.sync.dma_start(
                out_d[:, :].rearrange("(ib p) c -> p ib c", ib=NIB),
                oall[:])

    nc.finalize()
    return nc


def _host_prep(inputs):
    """Per-core in_maps: slicing / layout transforms / dtype casts only."""
    import ml_dtypes
    bf = ml_dtypes.bfloat16
    f32 = np.float32

    node = inputs["node_fts"].astype(f32)
    edge = inputs["edge_fts"].astype(f32)
    graph = inputs["graph_fts"].astype(f32)
    adj = inputs["adj_mat"]

    ae_w = inputs["ae_w"].astype(f32)
    bd = np.zeros((128, 128), f32)
    for h in range(H):
        for j8 in range(8):
            bd[16 * j8:16 * j8 + FE, 8 * h + j8] = ae_w[:, h]
    cmb = np.zeros((128, 128), f32)          # only rows 0..24 used
    for h in range(H):
        for j8 in range(8):
            cmb[j8, 8 * h + j8] = -1.0e9     # mask rows
        cmb[8 + h, 8 * h:8 * h + 8] = 1.0    # att1 spread rows
    ti = np.zeros((128, 128), f32)           # rows 0..16 used
    for h in range(H):
        ti[h, 8 * h:8 * h + 8] = 1.0
    onesp = np.zeros((128, H), f32)
    for h in range(H):
        onesp[8 * h:8 * h + 8, h] = 1.0
    hbsum_h = (inputs["a1_b"] + inputs["a2_b"] + inputs["ae_b"]
               + inputs["ag_b"]).astype(f32)            # [H]
    hbsum = np.repeat(hbsum_h, 8)[:, None].astype(f32)  # p = 8h+j8

    cbf = np.zeros((128, CBF_COLS), f32)
    cbf[:, C_MW:C_MW + OUT] = inputs["m_w"].astype(f32)
    cbf[:, C_SW:C_SW + OUT] = inputs["skip_w"].astype(f32)
    cbf[:, C_A12:C_A12 + 2 * H] = np.concatenate(
        [inputs["a1_w"], inputs["a2_w"]], 1).astype(f32)
    cbf[:, C_AGW:C_AGW + H] = inputs["ag_w"].astype(f32)
    cbf[:, C_BD:C_BD + 128] = bd
    cbf[:, C_CMB:C_CMB + 128] = cmb
    cbf[:, C_TI:C_TI + 128] = ti
    cbf[:, C_ONES:C_ONES + H] = onesp
    for j8 in range(8):
        t8 = np.zeros((128, 128), f32)      # rows 0..16 used: h -> (8h+j8)
        for h in range(H):
            t8[h, 8 * h + j8] = 1.0
        cbf[:, C_TI8 + 128 * j8:C_TI8 + 128 * (j8 + 1)] = t8

    cf32 = np.zeros((128, CF32_COLS), f32)
    cf32[:, F_MB:F_MB + OUT] = np.broadcast_to(
        (inputs["m_b"] + inputs["skip_b"]).astype(f32), (128, OUT))
    cf32[:, F_SB:F_SB + OUT] = np.broadcast_to(inputs["skip_b"].astype(f32),
                                               (128, OUT))
    cf32[:, F_LNS:F_LNS + OUT] = np.broadcast_to(
        inputs["ln_scale"].astype(f32), (128, OUT))
    cf32[:, F_LNO:F_LNO + OUT] = np.broadcast_to(
        inputs["ln_offset"].astype(f32), (128, OUT))
    cf32[:, F_HB] = hbsum[:, 0]
    cf32[:, F_EPS] = 1e-5

    in_maps = []
    for c in range(NCORES):
        b, half = c // 2, c % 2
        i0 = half * ROWS
        nadjT = np.ascontiguousarray(
            (1.0 - adj[b].astype(f32)).T[:, i0:i0 + ROWS])     # [j, i]
        nadj2 = np.ascontiguousarray(
            nadjT.reshape(NJB, NCC, 8, ROWS).transpose(2, 0, 1, 3)
        ).reshape(8, NCH, ROWS)
        # edge [i, j, f] -> [jb, (j8 f), cc, i]  (j = 128*jb + 8*cc + j8)
        ept = edge[b, i0:i0 + ROWS].reshape(ROWS, NJB, NCC, 8, FE)
        ept = np.ascontiguousarray(ept.transpose(1, 3, 4, 2, 0)).astype(
            ml_dtypes.float8_e4m3fn)
        cb = cbf.copy()
        cb[:, C_GT] = graph[b]
        cb[:, C_NODET:C_NODET + N] = node[b].T
        cb[:, C_NOWN:C_NOWN + ROWS] = node[b].T[:, i0:i0 + ROWS]
        cf_ = cf32.copy()
        cf_[:, F_GT] = graph[b]
        m = {
            "edge": ept.reshape(NJB, 128, NCC, ROWS),
            "nadj2": nadj2.astype(bf),
            "cbf": cb.astype(bf),
            "cf32": cf_,
        }
        in_maps.append(m)
    return in_maps


def run_device(inputs, want_results=True, **kw):
    """Compile (cached) + run on 8 cores. Returns (full_output, results)."""
    from concourse.bass_utils import run_bass_kernel_spmd
    if "nc" not in _CACHED:
        _CACHED["nc"] = _build_module()
    nc = _CACHED["nc"]
    in_maps = _host_prep(inputs)
    res = run_bass_kernel_spmd(nc, in_maps, core_ids=list(range(NCORES)), **kw)
    full = np.empty((B, N, OUT), dtype=np.float32)
    for c in range(NCORES):
        b, half = c // 2, c % 2
        full[b, half * ROWS:(half + 1) * ROWS] = res.results[c]["out"]
    return full, res


def kernel(**inputs):
    inputs = {k: np.asarray(v) for k, v in inputs.items()}
    out, _ = run_device(inputs)
    return out


# revision 10
# speedup vs baseline: 1.0327x; 1.0327x over previous
"""GAT layer kernel v3 for Trainium2 — nn_Basic_GAT_80874234184376.

Sharding (8 cores): graph b = core//2, query-row half i0 = (core%2)*512.

Per core (i: 512 query rows, j: 1024 keys, h: 16 heads, chunk: 8 keys):
  - edge pre-transposed + bf16 on HOST to [jb, (j8,f), cc, i]; 1 DMA/j-block.
  - psl[(h,j8), i] = CMB matmul (mask -1e9 rows + att1 spread rows against a
    staged [24,16,512] rhs of nadj-chunk rows and replicated att1T)
    += BD matmul (block-diag ae_w) @ edge-slab slice (att_e).
    att2 + biases + att_g ride in as per-partition activation bias.
  - softmax-invariant scaling by 100: E = max(100*exp(y), y+100)
    = ACT Exp(bias+ln100) then ONE fused DVE scalar_tensor_tensor.
  - U[i, 272] += E.T @ vbig (resident values + ones cols, bf16), 2-chunk lag.
  - epilogue: U/den, + skip matmul + skip_b, relu, layernorm; DVE/Pool split.
HWDGE descriptor gen is ~630ns serial per DMA -> constants packed into two
host buffers (1 DMA each), single-DMA gathers for att2/vald roundtrips.
"""

import numpy as np

B, N, FN, FE, FG = 4, 1024, 128, 16, 128
OUT, H = 256, 16
HD = OUT // H
NCORES = 8
ROWS = N // 2          # query rows per core
NIB = ROWS // 128      # 4 i-blocks
NJB = N // 8 // 16     # 8 j-blocks
NCC = 16               # 8-key chunks per j-block
NCH = NJB * NCC        # 128 chunks total

import os as _os
DBG_NJB = int(_os.environ.get("DBG_NJB", str(NJB)))
DBG_NO_EPI = _os.environ.get("DBG_NO_EPI", "0") == "1"

# packed bf16 const buffer column offsets
C_NODET, C_NOWN, C_MW, C_SW = 0, 1024, 1536, 1792
C_A12, C_AGW, C_BD, C_CMB, C_TI, C_ONES = 2048, 2080, 2096, 2224, 2352, 2480
C_TI8 = 2496
C_GT = 2496 + 8 * 128
CBF_COLS = C_GT + 1
# packed f32 const buffer column offsets
F_MB, F_SB, F_LNS, F_LNO, F_HB, F_GT, F_EPS = 0, 256, 512, 768, 1024, 1025, 1026
CF32_COLS = 1027

_CACHED = {}


def _patched_tc(nc):
    """TileContext whose tail drain splits multi-sem waits (this walrus build
    allows at most one sem wait per non-EVSEM instruction)."""
    import concourse.mybir as mybir
    from concourse.tile import TileContext
    from concourse.vector_clock import ScopedClock

    class TC(TileContext):
        def _drain_and_barrier(self, tick_clock, wait_clock):
            ncc = self.nc
            drain_inst = ncc.sync.drain()
            wait_clock.add_sem_waits(
                drain_inst.ins, ScopedClock({None: tick_clock.global_clock}))
            si = drain_inst.ins.sync_info
            waits = list(si.on_wait) if si is not None else []
            if len(waits) > 1:
                drain_inst.ins.sync_info = mybir.SyncInfo(
                    on_wait=[waits[0]], on_update=list(si.on_update))
                for w in waits[1:]:
                    extra = ncc.sync.drain()
                    extra.ins.sync_info = mybir.SyncInfo(on_wait=[w], on_update=[])
            ncc.all_engine_barrier()
            popped = ncc._tile_sem_poison_stack.pop()
            assert popped is self._sem_poison
            ncc.clear_and_free_semaphores(list(self.sems.allocated().values()))
            ncc.all_engine_barrier()

    return TC(nc)


def _build_module():
    import concourse.bacc as bacc
    import concourse.mybir as mybir

    f32 = mybir.dt.float32
    bf16 = mybir.dt.bfloat16
    fp8 = mybir.dt.float8e4
    AL = mybir.AluOpType
    ACT = mybir.ActivationFunctionType

    nc = bacc.Bacc("TRN2", num_devices=NCORES)

    edge_d = nc.dram_tensor("edge", (NJB, 128, NCC, ROWS), fp8,
                            kind="ExternalInput")
    nadj2_d = nc.dram_tensor("nadj2", (8, NCH, ROWS), bf16,
                             kind="ExternalInput")
    cbf_d = nc.dram_tensor("cbf", (128, CBF_COLS), bf16, kind="ExternalInput")
    cf32_d = nc.dram_tensor("cf32", (128, CF32_COLS), f32,
                            kind="ExternalInput")
    out_d = nc.dram_tensor("out", (ROWS, OUT), f32, kind="ExternalOutput")

    vald = nc.dram_tensor("vald", (N, OUT), bf16)      # internal scratch
    att1d = nc.dram_tensor("att1d", (H, ROWS), bf16)   # internal scratch

    with _patched_tc(nc) as tc:
        with (
            tc.tile_pool(name="const", bufs=1) as cpool,
            tc.tile_pool(name="work", bufs=2) as wpool,
            tc.tile_pool(name="ew", bufs=int(_os.environ.get("EWB", "4"))) as ewpool,
            tc.tile_pool(name="epool", bufs=8) as eppool,
            tc.tile_pool(name="upsum", bufs=1, space="PSUM") as upool,
        ):
            # ---- bulk const loads (2 DMAs) + slabs + nadj (3 DMAs) -------
            cbf = cpool.tile([128, CBF_COLS], bf16)
            nc.sync.dma_start(cbf[:, N:], cbf_d[:, N:])
            nc.sync.dma_start(cbf[:, 0:N], cbf_d[:, 0:N])
            cf = cpool.tile([128, CF32_COLS], f32)
            nc.sync.dma_start(cf[:], cf32_d[:, :])
            esbs = [cpool.tile([128, NCC, ROWS], fp8, name=f"esb{k}")
                    for k in range(3)]
            nc.sync.dma_start(esbs[0][:, 0:4], edge_d[0, :, 0:4])
            nc.sync.dma_start(esbs[0][:, 4:16], edge_d[0, :, 4:16])

            nodeT = cbf[:, C_NODET:C_NODET + N]
            nown = cbf[:, C_NOWN:C_NOWN + ROWS]
            mw_sb = cbf[:, C_MW:C_MW + OUT]
            sw_sb = cbf[:, C_SW:C_SW + OUT]
            a12_sb = cbf[:, C_A12:C_A12 + 2 * H]
            agw_sb = cbf[:, C_AGW:C_AGW + H]
            bd_sb = cbf[:, C_BD:C_BD + 128]
            cmb_sb = cbf[0:24, C_CMB:C_CMB + 128]
            ti_sb = cbf[0:16, C_TI:C_TI + 128]
            ones_sb = cbf[:, C_ONES:C_ONES + H]
            mb_sb = cf[:, F_MB:F_MB + OUT]
            sb_sb = cf[:, F_SB:F_SB + OUT]
            lns_sb = cf[:, F_LNS:F_LNS + OUT]
            lno_sb = cf[:, F_LNO:F_LNO + OUT]
            hb_sb = cf[:, F_HB:F_HB + 1]
            gt_sb = cf[:, F_GT:F_GT + 1]
            epst = cf[:, F_EPS:F_EPS + 1]

            # ---- vbig: double-buffered per-j-block values ----------------
            VG = 4
            vbufs = [cpool.tile([128, VG, NCC, OUT + H], bf16, name=f"vb{k}")
                     for k in range(2)]
            # zeros+ones: buf0 on ACT+DVE (needed early), buf1 on Pool
            nc.scalar.activation(
                vbufs[0][:].rearrange("p g b c -> p (g b c)"),
                ones_sb[:, 0:1].broadcast_to((128, VG * NCC * (OUT + H))),
                ACT.Identity, scale=0.0)
            nc.vector.tensor_copy(
                vbufs[0][:, :, :, OUT:OUT + H],
                ones_sb[:, None, None, :].broadcast_to((128, VG, NCC, H)))
            nc.vector.memset(vbufs[1][:, 0:2], 0.0)
            nc.gpsimd.memset(vbufs[1][:, 2:4], 0.0)
            nc.gpsimd.tensor_copy(
                vbufs[1][:, :, :, OUT:OUT + H],
                ones_sb[:, None, None, :].broadcast_to((128, VG, NCC, H)))

            def refill_vbig(g, split=False):
                for h in range(H):
                    nc.sync.dma_start(
                        vbufs[g % 2][8 * h:8 * h + 8, :, :,
                                     16 * h:16 * (h + 1)],
                        vald[g * VG * 128:(g + 1) * VG * 128,
                             16 * h:16 * (h + 1)]
                        .rearrange("(g c j8) hd -> j8 g c hd", j8=8, c=NCC))

            att1T = cpool.tile([H, ROWS], bf16)
            att2e = cpool.tile([128, NCH], f32)   # att2 bias + ln(100)
            att2l = cpool.tile([128, NCH], f32)   # att2 bias + 100
            stags = [cpool.tile([24, NCC, ROWS], bf16, name=f"stag{k}")
                     for k in range(3)]

            with (tc.tile_pool(name="p0ps", bufs=1, space="PSUM") as p0ps,
                  tc.tile_pool(name="p0d", bufs=2, space="PSUM") as p0d):
                # att_g: (graph @ ag_w) spread to 128 partitions + host biases
                gps = p0ps.tile([128, 512], f32, tag="pS", name="gps")
                nc.tensor.matmul(gps[0:H, 0:1], agw_sb[:],
                                 cbf[:, C_GT:C_GT + 1], start=True,
                                 stop=True)
                gsb = wpool.tile([H, 1], bf16, tag="gsb")
                nc.vector.tensor_copy(gsb[:], gps[0:H, 0:1])
                gx = p0ps.tile([128, 512], f32, tag="pS", name="gx")
                nc.tensor.matmul(gx[:, 0:1], ti_sb, gsb[:], start=True, stop=True)
                hb128 = cpool.tile([128, 1], f32)
                nc.vector.tensor_tensor(hb128[:], gx[:, 0:1], hb_sb, AL.add)

                # att1T directly transposed: [h, i] = a1_w.T @ nownT
                a1ps = p0ps.tile([128, ROWS], f32, tag="pS", name="a1ps")
                nc.tensor.matmul(a1ps[0:H, :], a12_sb[:, 0:H], nown[:],
                                 start=True, stop=True)
                nc.vector.tensor_copy(att1T[:], a1ps[0:H, :])
                nc.sync.dma_start(att1d[:, :], att1T[:])
                for k in range(3):
                    nc.sync.dma_start(
                        stags[k][8:24, :, :],
                        att1d[:, None, :].broadcast_to((H, NCC, ROWS)))
                for k in range(min(2, DBG_NJB)):
                    nc.sync.dma_start(stags[k][0:8, :, :],
                                      nadj2_d[:, k * NCC:(k + 1) * NCC, :])

                # values = node @ m_w + (m_b + skip_b bias) -> vald
                vall = cpool.tile([128, 8, OUT], bf16)
                for jt in range(8):
                    vps = p0d.tile([128, OUT], f32, tag="pD", name=f"v{jt}")
                    nc.tensor.matmul(vps[:], nodeT[:, jt * 128:(jt + 1) * 128],
                                     mw_sb[:], start=True, stop=True)
                    nc.vector.tensor_tensor(vall[:, jt, :], vps[:], mb_sb[:],
                                            AL.add)
                nc.sync.dma_start(
                    vald[:, :].rearrange("(jt p) c -> p jt c", jt=8),
                    vall[:])
                for h in range(H):
                    eng_q = nc.sync if h % 2 else nc.gpsimd
                    eng_q.dma_start(
                        vbufs[0][8 * h:8 * h + 8, :, :, 16 * h:16 * (h + 1)],
                        vald[0:VG * 128, 16 * h:16 * (h + 1)]
                        .rearrange("(g c j8) hd -> j8 g c hd", j8=8, c=NCC))
                if DBG_NJB > 1:
                    nc.sync.dma_start(esbs[1][:], edge_d[1])
                if DBG_NJB > 2:
                    nc.sync.dma_start(esbs[2][:], edge_d[2])
                # att2T directly transposed: [h, j] = a2_w.T @ nodeT
                att2T = cpool.tile([H, N], bf16)
                for half in range(2):
                    a2ps = p0ps.tile([128, ROWS], f32, tag="pS",
                                     name=f"a2ps{half}")
                    nc.tensor.matmul(a2ps[0:H, :],
                                     a12_sb[:, H:2 * H],
                                     nodeT[:, half * ROWS:(half + 1) * ROWS],
                                     start=True, stop=True)
                    nc.vector.tensor_copy(
                        att2T[:, half * ROWS:(half + 1) * ROWS], a2ps[0:H, :])
                # partition-spread att2[j, h] -> a2xps[(h,j8), cg] in PSUM
                a2xps = p0ps.tile([128, NCH], f32, tag="pC", name="a2xps")
                for j8 in range(8):
                    nc.tensor.matmul(
                        a2xps[:],
                        cbf[0:16, C_TI8 + 128 * j8:C_TI8 + 128 * (j8 + 1)],
                        att2T[:, :].rearrange("h (cg j8) -> h cg j8",
                                              j8=8)[:, :, j8],
                        start=(j8 == 0), stop=(j8 == 7))
                a2xp = wpool.tile([128, NCH], f32, tag="a2xp")
                nc.vector.tensor_scalar(a2xp[:], a2xps[:], hb128[:, 0:1], None,
                                        AL.add)
                nc.vector.tensor_scalar(att2e[:], a2xp[:], 4.605170185988091,
                                        None, AL.add)
                nc.vector.tensor_scalar(att2l[:], a2xp[:], 100.0, None,
                                        AL.add)

                sksb = cpool.tile([128, NIB, OUT], bf16)
                for ib in range(NIB):
                    skp = p0d.tile([128, OUT], f32, tag="pD", name=f"sk{ib}")
                    nc.tensor.matmul(skp[:], nown[:, ib * 128:(ib + 1) * 128],
                                     sw_sb[:], start=True, stop=True)
                    nc.vector.tensor_copy(sksb[:, ib, :], skp[:])

            # U accumulators: [i(128), 256 vals + 16 den] per i-block
            upsums = [upool.tile([128, OUT + H], f32, tag=f"U{ib}",
                                 name=f"U{ib}") for ib in range(NIB)]

            # ---------------- main loop ----------------
            LAG = int(_os.environ.get('LAG', '4'))
            WARM = min(int(_os.environ.get('WARM', '10')), max(2, DBG_NJB * NCC - 2))
            NCG = DBG_NJB * NCC
            etiles = {}
            unext = [0]

            def emit_u(cg2):
                jb2, cc2 = cg2 // NCC, cg2 % NCC
                E2 = etiles.pop(cg2)
                for ib in range(NIB):
                    nc.tensor.matmul(
                        upsums[ib][:],
                        E2[:, ib * 128:(ib + 1) * 128],
                        vbufs[(jb2 // VG) % 2][:, jb2 % VG, cc2, :],
                        start=(cg2 == 0), stop=(cg2 == NCG - 1))

            with tc.tile_pool(name="pslp", bufs=4, space="PSUM") as pslpool:
                for jb in range(DBG_NJB):
                    esb = esbs[jb % 3]
                    stag = stags[jb % 3]
                    for cc in range(NCC):
                        cg = jb * NCC + cc
                        if cg >= WARM + LAG:
                            n = 2 if unext[0] < cg - LAG - 1 else 1
                            for _ in range(n):
                                if unext[0] <= cg - LAG - 1:
                                    emit_u(unext[0])
                                    unext[0] += 1
                        psl = pslpool.tile([128, ROWS], f32, tag="psl")
                        nc.tensor.matmul(psl[:], cmb_sb, stag[:, cc, :],
                                         start=True, stop=False)
                        nc.tensor.matmul(psl[:], bd_sb, esb[:, cc, :],
                                         start=False, stop=True)
                        e0 = ewpool.tile([128, ROWS], bf16, tag="e0")
                        nc.scalar.activation(e0[:], psl[:], ACT.Exp,
                                             bias=att2e[:, cg:cg + 1],
                                             scale=1.0)
                        E = eppool.tile([128, ROWS], bf16, tag="E")
                        nc.vector.scalar_tensor_tensor(
                            E[:], psl[:], att2l[:, cg:cg + 1], e0[:],
                            AL.add, AL.max)
                        etiles[cg] = E
                        if cc == 2 and jb % VG == 1 and (jb // VG) + 1 < (DBG_NJB + VG - 1) // VG:
                            refill_vbig(jb // VG + 1)

                    if jb + 2 < DBG_NJB:
                        nc.sync.dma_start(
                            stags[(jb + 2) % 3][0:8, :, :],
                            nadj2_d[:, (jb + 2) * NCC:(jb + 3) * NCC, :])
                    if jb + 3 < DBG_NJB:
                        nc.sync.dma_start(esbs[(jb + 3) % 3][:],
                                          edge_d[jb + 3])
                for cg in range(unext[0], NCG):
                    emit_u(cg)

            # ---------------- epilogue ----------------
            oall = cpool.tile([128, NIB, OUT], f32)
            NEP = 0 if DBG_NO_EPI else NIB
            recs = [wpool.tile([128, H, 1], f32, tag=f"rec{i}", name=f"rec{i}") for i in range(NEP)]
            unos = [wpool.tile([128, OUT], bf16, tag=f"uno{i}", name=f"uno{i}") for i in range(NEP)]
            mus = [wpool.tile([128, 1], f32, tag=f"mu{i}", name=f"mu{i}") for i in range(NEP)]
            cens = [wpool.tile([128, OUT], bf16, tag=f"cen{i}", name=f"cen{i}") for i in range(NEP)]
            sqs = [wpool.tile([128, OUT], bf16, tag=f"sq{i}", name=f"sq{i}") for i in range(NEP)]
            vars_ = [wpool.tile([128, 1], f32, tag=f"var{i}", name=f"var{i}") for i in range(NEP)]
            stds = [wpool.tile([128, 1], f32, tag=f"std{i}", name=f"std{i}") for i in range(NEP)]
            rstds = [wpool.tile([128, 1], f32, tag=f"rst{i}", name=f"rst{i}") for i in range(NEP)]
            engs = [nc.vector if i == 3 else nc.gpsimd for i in range(NEP)]
            for ib in range(NEP):
                nc.vector.reciprocal(recs[ib][:, :, 0],
                                     upsums[ib][:, OUT:OUT + H])
            for ib in range(NEP):
                nc.vector.tensor_tensor(
                    unos[ib][:].rearrange("p (h d) -> p h d", h=H),
                    upsums[ib][:, 0:OUT].rearrange("p (h d) -> p h d", h=H),
                    recs[ib][:, :, :].broadcast_to((128, H, HD)),
                    AL.mult)
            for ib in range(NEP):
                engs[ib].tensor_tensor(unos[ib][:], unos[ib][:],
                                       sksb[:, ib, :], AL.add)
            for ib in range(NEP):
                engs[ib].tensor_scalar(unos[ib][:], unos[ib][:], 0.0, None,
                                       AL.max)
            for ib in range(NEP):
                nc.vector.reduce_sum(mus[ib][:], unos[ib][:],
                                     axis=mybir.AxisListType.X)
            for ib in range(NEP):
                engs[ib].tensor_scalar(mus[ib][:], mus[ib][:], 1.0 / OUT,
                                       None, AL.mult)
            for ib in range(NEP):
                nc.vector.tensor_scalar(cens[ib][:], unos[ib][:],
                                        mus[ib][:, 0:1], None, AL.subtract)
            for ib in range(NEP):
                engs[ib].tensor_tensor(sqs[ib][:], cens[ib][:], cens[ib][:],
                                       AL.mult)
            for ib in range(NEP):
                nc.vector.reduce_sum(vars_[ib][:], sqs[ib][:],
                                     axis=mybir.AxisListType.X)
            for ib in range(NEP):
                nc.scalar.activation(stds[ib][:], vars_[ib][:], ACT.Sqrt,
                                     bias=epst, scale=1.0 / OUT)
            for ib in range(NEP):
                nc.vector.reciprocal(rstds[ib][:], stds[ib][:])
            for ib in range(NEP):
                nc.vector.scalar_tensor_tensor(cens[ib][:], cens[ib][:],
                                               rstds[ib][:, 0:1], lns_sb,
                                               AL.mult, AL.mult)
            for ib in range(NEP):
                engs[ib].tensor_tensor(oall[:, ib, :], cens[ib][:], lno_sb,
                                       AL.add)
            nc.sync.dma_start(
                out_d[:, :].rearrange("(ib p) c -> p ib c", ib=NIB),
                oall[:])

    nc.finalize()
    return nc


def _host_prep(inputs):
    """Per-core in_maps: slicing / layout transforms / dtype casts only."""
    import ml_dtypes
    bf = ml_dtypes.bfloat16
    f32 = np.float32

    node = inputs["node_fts"].astype(f32)
    edge = inputs["edge_fts"].astype(f32)
    graph = inputs["graph_fts"].astype(f32)
    adj = inputs["adj_mat"]

    ae_w = inputs["ae_w"].astype(f32)
    bd = np.zeros((128, 128), f32)
    for h in range(H):
        for j8 in range(8):
            bd[16 * j8:16 * j8 + FE, 8 * h + j8] = ae_w[:, h]
    cmb = np.zeros((128, 128), f32)          # only rows 0..24 used
    for h in range(H):
        for j8 in range(8):
            cmb[j8, 8 * h + j8] = -1.0e9     # mask rows
        cmb[8 + h, 8 * h:8 * h + 8] = 1.0    # att1 spread rows
    ti = np.zeros((128, 128), f32)           # rows 0..16 used
    for h in range(H):
        ti[h, 8 * h:8 * h + 8] = 1.0
    onesp = np.zeros((128, H), f32)
    for h in range(H):
        onesp[8 * h:8 * h + 8, h] = 1.0
    hbsum_h = (inputs["a1_b"] + inputs["a2_b"] + inputs["ae_b"]
               + inputs["ag_b"]).astype(f32)            # [H]
    hbsum = np.repeat(hbsum_h, 8)[:, None].astype(f32)  # p = 8h+j8

    cbf = np.zeros((128, CBF_COLS), f32)
    cbf[:, C_MW:C_MW + OUT] = inputs["m_w"].astype(f32)
    cbf[:, C_SW:C_SW + OUT] = inputs["skip_w"].astype(f32)
    cbf[:, C_A12:C_A12 + 2 * H] = np.concatenate(
        [inputs["a1_w"], inputs["a2_w"]], 1).astype(f32)
    cbf[:, C_AGW:C_AGW + H] = inputs["ag_w"].astype(f32)
    cbf[:, C_BD:C_BD + 128] = bd
    cbf[:, C_CMB:C_CMB + 128] = cmb
    cbf[:, C_TI:C_TI + 128] = ti
    cbf[:, C_ONES:C_ONES + H] = onesp
    for j8 in range(8):
        t8 = np.zeros((128, 128), f32)      # rows 0..16 used: h -> (8h+j8)
        for h in range(H):
            t8[h, 8 * h + j8] = 1.0
        cbf[:, C_TI8 + 128 * j8:C_TI8 + 128 * (j8 + 1)] = t8

    cf32 = np.zeros((128, CF32_COLS), f32)
    cf32[:, F_MB:F_MB + OUT] = np.broadcast_to(
        (inputs["m_b"] + inputs["skip_b"]).astype(f32), (128, OUT))
    cf32[:, F_SB:F_SB + OUT] = np.broadcast_to(inputs["skip_b"].astype(f32),
                                               (128, OUT))
    cf32[:, F_LNS:F_LNS + OUT] = np.broadcast_to(
        inputs["ln_scale"].astype(f32), (128, OUT))
    cf32[:, F_LNO:F_LNO + OUT] = np.broadcast_to(
        inputs["ln_offset"].astype(f32), (128, OUT))
    cf32[:, F_HB] = hbsum[:, 0]
    cf32[:, F_EPS] = 1e-5

    in_maps = []
    for c in range(NCORES):
        b, half = c // 2, c % 2
        i0 = half * ROWS
        nadjT = np.ascontiguousarray(
            (1.0 - adj[b].astype(f32)).T[:, i0:i0 + ROWS])     # [j, i]
        nadj2 = np.ascontiguousarray(
            nadjT.reshape(NJB, NCC, 8, ROWS).transpose(2, 0, 1, 3)
        ).reshape(8, NCH, ROWS)
        # edge [i, j, f] -> [jb, (j8 f), cc, i]  (j = 128*jb + 8*cc + j8)
        ept = edge[b, i0:i0 + ROWS].reshape(ROWS, NJB, NCC, 8, FE)
        ept = np.ascontiguousarray(ept.transpose(1, 3, 4, 2, 0)).astype(
            ml_dtypes.float8_e4m3fn)
        cb = cbf.copy()
        cb[:, C_GT] = graph[b]
        cb[:, C_NODET:C_NODET + N] = node[b].T
        cb[:, C_NOWN:C_NOWN + ROWS] = node[b].T[:, i0:i0 + ROWS]
        cf_ = cf32.copy()
        cf_[:, F_GT] = graph[b]
        m = {
            "edge": ept.reshape(NJB, 128, NCC, ROWS),
            "nadj2": nadj2.astype(bf),
            "cbf": cb.astype(bf),
            "cf32": cf_,
        }
        in_maps.append(m)
    return in_maps


def run_device(inputs, want_results=True, **kw):
    """Compile (cached) + run on 8 cores. Returns (full_output, results)."""
    from concourse.bass_utils import run_bass_kernel_spmd
    if "nc" not in _CACHED:
        _CACHED["nc"] = _build_module()
    nc = _CACHED["nc"]
    in_maps = _host_prep(inputs)
    res = run_bass_kernel_spmd(nc, in_maps, core_ids=list(range(NCORES)), **kw)
    full = np.empty((B, N, OUT), dtype=np.float32)
    for c in range(NCORES):
        b, half = c // 2, c % 2
        full[b, half * ROWS:(half + 1) * ROWS] = res.results[c]["out"]
    return full, res


def kernel(**inputs):
    inputs = {k: np.asarray(v) for k, v in inputs.items()}
    out, _ = run_device(inputs)
    return out
